# revision 1
# baseline (speedup 1.0000x reference)
"""NeRF MLP forward on 8 Trainium2 NeuronCores (Bass/Tile).

Data-parallel: the 131072-point batch is split into 8 shards of 16384.
On-device layout is feature-major ([features on partitions, batch on
free dim]); the host transposes x per shard and packs all weights into
a single [128, COLS] tensor whose 128-column blocks are matmul lhsT
tiles (K zero-padded to 128 so every matmul runs K=128 / N=512).
"""

import sys

import numpy as np

for _p in ("/opt/trn_rl_repo",):
    if _p not in sys.path:
        sys.path.append(_p)

N_TOTAL = 131072
NCORES = 8
BCORE = N_TOTAL // NCORES  # 16384 points per core
NB = 512                   # batch tile (one PSUM bank of fp32)
IN_CH = 63
UNITS = 256


def _col_layout():
    cols = {}
    cur = 0

    def alloc(name, n):
        nonlocal cur
        cols[name] = cur
        cur += n

    alloc("W0", 256)                       # 1 K-block x 2 M-tiles
    for l in range(1, 8):
        alloc(f"W{l}", (3 if l == 5 else 2) * 256)
    # feature layer folded: Wfv = Wf @ Wv[0:256]  (feature only feeds Wv)
    alloc("Wv", 384)                       # 3 K-blocks x 1 M-tile(128)
    alloc("Wh", 12)                        # 3 K-blocks x M=4
    alloc("bb", 16)                        # backbone biases, [128] halves
    alloc("bv", 1)
    alloc("bh", 1)
    return cols, cur


COLS_MAP, COLS = _col_layout()


def _pack_weights(inp):
    w = np.zeros((128, COLS), np.float32)
    c = COLS_MAP
    w[0:63, c["W0"]:c["W0"] + 256] = inp["W0"]
    for l in (1, 2, 3, 4, 6, 7):
        b = c[f"W{l}"]
        W = inp[f"W{l}"]
        w[:, b:b + 256] = W[0:128]
        w[:, b + 256:b + 512] = W[128:256]
    b = c["W5"]
    W5 = inp["W5"]                          # rows 0:63 pts, 63:319 h
    w[0:63, b:b + 256] = W5[0:63]
    w[:, b + 256:b + 512] = W5[63:191]
    w[:, b + 512:b + 768] = W5[191:319]
    b = c["Wv"]
    Wv = inp["Wv"]                          # rows 0:256 feature, 256:319 views
    Wfv = (inp["Wf"].astype(np.float64) @ Wv[0:256].astype(np.float64)
           ).astype(np.float32)
    w[:, b:b + 128] = Wfv[0:128]
    w[:, b + 128:b + 256] = Wfv[128:256]
    w[63:126, b + 256:b + 384] = Wv[256:319]
    b = c["Wh"]
    w[:, b + 0:b + 3] = inp["Wr"]           # rgb rows from hv
    w[:, b + 7:b + 8] = inp["Wa"][0:128]    # alpha from h half0
    w[:, b + 11:b + 12] = inp["Wa"][128:256]
    b = c["bb"]
    for l in range(8):
        bl = inp[f"b{l}"]
        w[:, b + 2 * l] = bl[0:128]
        w[:, b + 2 * l + 1] = bl[128:256]
    # feature bias folded into the view-layer bias: bv' = bv + bf @ Wv[:256]
    w[:, c["bv"]] = (inp["bv"].astype(np.float64)
                     + inp["bf"].astype(np.float64)
                     @ inp["Wv"][0:256].astype(np.float64)).astype(np.float32)
    w[0:3, c["bh"]] = inp["br"]
    w[3, c["bh"]] = inp["ba"][0]
    return w


def build_nc(bcore=BCORE, mm_mode="f32r", repeats=1, group=3, pp_main=None):
    import concourse.bacc as bacc
    import concourse.bass as bass
    import concourse.mybir as mybir
    import concourse.tile as tile

    f32 = mybir.dt.float32
    # mdt: dtype for all matmul operands (DRAM + SBUF activations/weights).
    # float32r = fp32 bit layout, single-pass PE matmul (4x faster than f32);
    # engines round when writing it, and the BIR verifier requires operands
    # of an f32r matmul to be produced as f32r.
    mdt = {"f32r": mybir.dt.float32r, "f32": f32,
           "bf16": mybir.dt.bfloat16}[mm_mode]
    AF = mybir.ActivationFunctionType
    OP = mybir.AluOpType
    c = COLS_MAP
    nt = bcore // NB

    # Bacc (not Bass): its finalize() runs generate_event_semaphores, which
    # splits multi-sem waits — TRN2 codegen allows only 1 sync wait per inst.
    nc = bacc.Bacc("TRN2", target_bir_lowering=False, debug=False)
    xt_d = nc.declare_dram_parameter("xt", [128, bcore], mdt, False)
    w_d = nc.declare_dram_parameter("wall", [128, COLS], mdt, False)
    out_d = nc.declare_dram_parameter("out", [4, bcore], f32, True)

    def bias_cast(ap):
        return ap.bitcast(f32) if mdt == mybir.dt.float32r else ap

    with tile.TileContext(nc) as tc:
        with (
            tc.tile_pool(name="wp", bufs=3) as wp,
            tc.tile_pool(name="xp", bufs=2 * group) as xp,
            tc.tile_pool(name="hp", bufs=2 * group) as hp,
            tc.tile_pool(name="vp", bufs=group) as vp,
            tc.tile_pool(name="op", bufs=2 * group) as op,
            tc.tile_pool(name="pp",
                         bufs=pp_main or max(6, min(2 * group, 7)),
                         space=bass.MemorySpace.PSUM) as pp,
            tc.tile_pool(name="pp4",
                         bufs=8 - (pp_main or max(6, min(2 * group, 7))),
                         space=bass.MemorySpace.PSUM) as pp4,
        ):
            # weights in 3 separate tiles (one DMA queue each) so a matmul
            # depends on exactly one weight-load semaphore
            dma_engines = [nc.sync, nc.gpsimd, nc.scalar]
            edges = [0, c["W3"], c["W6"], COLS]
            w_tiles = []
            for i in range(3):
                c0, c1 = edges[i], edges[i + 1]
                wt = wp.tile([128, c1 - c0], mdt)
                dma_engines[i].dma_start(wt[:], w_d[:, c0:c1])
                w_tiles.append((c0, c1, wt))

            def wslice(col0, width, p=None):
                for (a, b, wt) in w_tiles:
                    if a <= col0 and col0 + width <= b:
                        if p is None:
                            return wt[:, col0 - a:col0 - a + width]
                        return wt[p[0]:p[1], col0 - a:col0 - a + width]
                raise AssertionError(f"col range {col0}+{width} spans chunks")

            def wtile(base, idx, m=128):
                return wslice(base + idx * m, m)

            # Two independent NB-wide sub-batches (A/B) interleaved per layer:
            # PE runs B's matmuls while A's PSUM halves drain through ACT/DVE,
            # hiding the drain latency that otherwise stalls PE each layer.
            def pair_body(ts):
                xrs = []
                for i, t in enumerate(ts):
                    x_t = xp.tile([128, NB], mdt)
                    dma_engines[i % 2].dma_start(
                        x_t[:], xt_d[:, t * NB:(t + 1) * NB])
                    xrs.append(x_t[:])

                # k-outermost issue order: a PSUM bank's start and stop
                # matmuls are separated by the other 2*group banks' matmuls.
                # Adjacent same-bank start/stop pairs measure 253.5 ns/mm on
                # HW; separated pairs measure 233.0 ns/mm (bench_pe grp2i).
                hs = [None] * len(ts)
                for l in range(8):
                    base = c[f"W{l}"]
                    nk = 3 if l == 5 else (1 if l == 0 else 2)
                    hns = [hp.tile([128, 2 * NB], mdt, name="hn")
                           for _ in ts]
                    pss = [[pp.tile([128, NB], f32, name="ps")
                            for _ in range(2)] for _ in ts]
                    for k in range(nk):
                        for s in range(len(ts)):
                            h = hs[s]
                            if l == 0:
                                rhs = xrs[s]
                            elif l == 5:
                                rhs = (xrs[s] if k == 0
                                       else h[:, (k - 1) * NB:k * NB])
                            else:
                                rhs = h[:, k * NB:(k + 1) * NB]
                            for m in range(2):
                                nc.tensor.matmul(
                                    pss[s][m][:], wtile(base, k * 2 + m),
                                    rhs, start=(k == 0), stop=(k == nk - 1),
                                    skip_group_check=True)
                    for s in range(len(ts)):
                        for m in range(2):
                            bias = bias_cast(wslice(c["bb"] + 2 * l + m, 1))
                            dst = hns[s][:, m * NB:(m + 1) * NB]
                            if m == 0:
                                nc.scalar.activation(dst, pss[s][m][:],
                                                     AF.Relu, bias=bias)
                            else:
                                nc.vector.tensor_scalar(dst, pss[s][m][:],
                                                        bias, 0.0,
                                                        OP.add, OP.max)
                        hs[s] = hns[s]

                vps = [pp.tile([128, NB], f32, name="ps") for _ in ts]
                for k in range(3):
                    for s in range(len(ts)):
                        h = hs[s]
                        rhs = xrs[s] if k == 2 else h[:, k * NB:(k + 1) * NB]
                        nc.tensor.matmul(vps[s][:], wtile(c["Wv"], k), rhs,
                                         start=(k == 0), stop=(k == 2),
                                         skip_group_check=True)
                hvs = []
                for s in range(len(ts)):
                    hv = vp.tile([128, NB], mdt)
                    nc.scalar.activation(hv[:], vps[s][:], AF.Relu,
                                         bias=bias_cast(wslice(c["bv"], 1)))
                    hvs.append(hv)

                for s, t in enumerate(ts):
                    h = hs[s]
                    ps4 = pp4.tile([4, NB], f32)
                    for k in range(3):
                        rhs = hvs[s][:] if k == 0 else h[:, (k - 1) * NB:k * NB]
                        nc.tensor.matmul(ps4[:], wtile(c["Wh"], k, m=4), rhs,
                                         start=(k == 0), stop=(k == 2))
                    ot = op.tile([4, NB], f32)
                    nc.vector.tensor_scalar_add(
                        ot[:], ps4[:], bias_cast(wslice(c["bh"], 1, p=(0, 4))))
                    nc.sync.dma_start(out_d[:, t * NB:(t + 1) * NB], ot[:])

            pairs = [tuple(range(j, min(j + group, nt)))
                     for j in range(0, nt, group)]

            if repeats > 1:
                # hardware loop: device re-runs the whole batch `repeats`
                # times in one dispatch (used only for timing via deltas)
                with tc.For_i(0, repeats):
                    for ts in pairs:
                        pair_body(ts)
            else:
                for ts in pairs:
                    pair_body(ts)

    nc.finalize()
    return nc


_NC_CACHE = {}


def _get_nc(mm_mode="f32r", repeats=1, group=3, pp_main=None):
    key = (mm_mode, repeats, group, pp_main)
    if key not in _NC_CACHE:
        _NC_CACHE[key] = build_nc(BCORE, mm_mode, repeats, group, pp_main)
    return _NC_CACHE[key]


def prepare(inputs):
    inp = {k: np.asarray(v, np.float32) for k, v in inputs.items()}
    wall = _pack_weights(inp)
    x = inp["x"]
    xt = np.zeros((NCORES, 128, BCORE), np.float32)
    for c in range(NCORES):
        xt[c, 0:126] = x[c * BCORE:(c + 1) * BCORE].T
    return [{"xt": xt[c], "wall": wall} for c in range(NCORES)]


def kernel(**inputs):
    from concourse.bass_utils import run_bass_kernel_spmd

    in_maps = prepare(inputs)
    nc = _get_nc("f32r", 1, 3, 6)
    res = run_bass_kernel_spmd(nc, in_maps, core_ids=list(range(NCORES)))
    out = np.empty((N_TOTAL, 4), np.float32)
    for c, r in enumerate(res.results):
        out[c * BCORE:(c + 1) * BCORE] = np.asarray(r["out"]).T
    return out


def make_runner(inputs, mm_mode="f32r", repeats=1, group=3, pp_main=None):
    """Build a reusable jitted executor for timing: one jit compile, inputs
    kept device-resident, fresh donated output buffers per call. Mirrors
    bass2jax.run_bass_via_pjrt's multi-core branch."""
    import jax
    from jax.experimental.shard_map import shard_map
    from jax.sharding import Mesh, NamedSharding, PartitionSpec

    import concourse.mybir as mybir
    from concourse.bass2jax import (_bass_exec_p, install_neuronx_cc_hook,
                                    partition_id_tensor)

    install_neuronx_cc_hook()
    nc = _get_nc(mm_mode, repeats, group, pp_main)
    in_maps = prepare(inputs)

    in_names, out_names, out_avals, zero_outs = [], [], [], []
    partition_name = nc.partition_id_tensor.name if nc.partition_id_tensor else None
    for alloc in nc.m.functions[0].allocations:
        if not isinstance(alloc, mybir.MemoryLocationSet):
            continue
        name = alloc.memorylocations[0].name
        if alloc.kind == "ExternalInput":
            if name != partition_name:
                in_names.append(name)
        elif alloc.kind == "ExternalOutput":
            shape = tuple(alloc.tensor_shape)
            dtype = mybir.dt.np(alloc.dtype)
            out_names.append(name)
            out_avals.append(jax.core.ShapedArray(shape, dtype))
            zero_outs.append(np.zeros(shape, dtype))
    n_params = len(in_names)
    n_outs = len(out_avals)
    all_names = list(in_names) + list(out_names)
    if partition_name is not None:
        all_names.append(partition_name)
    donate = tuple(range(n_params, n_params + n_outs))

    def _body(*args):
        operands = list(args)
        if partition_name is not None:
            operands.append(partition_id_tensor())
        return tuple(_bass_exec_p.bind(
            *operands,
            out_avals=tuple(out_avals),
            in_names=tuple(all_names),
            out_names=tuple(out_names),
            lowering_input_output_aliases=(),
            sim_require_finite=True,
            sim_require_nnan=True,
            nc=nc,
        ))

    devices = jax.devices()[:NCORES]
    mesh = Mesh(np.asarray(devices), ("core",))
    spec = NamedSharding(mesh, PartitionSpec("core"))
    sharded = jax.jit(
        shard_map(_body, mesh=mesh,
                  in_specs=(PartitionSpec("core"),) * (n_params + n_outs),
                  out_specs=(PartitionSpec("core"),) * n_outs,
                  check_rep=False),
        donate_argnums=donate, keep_unused=True)

    concat_in = [
        jax.device_put(
            np.concatenate([np.asarray(in_maps[c][nm]) for c in range(NCORES)], axis=0),
            spec)
        for nm in in_names
    ]

    def fresh_zeros():
        return [jax.device_put(np.zeros((NCORES * z.shape[0], *z.shape[1:]), z.dtype), spec)
                for z in zero_outs]

    def run(zeros=None):
        outs = sharded(*concat_in, *(zeros if zeros is not None else fresh_zeros()))
        jax.block_until_ready(outs)
        return outs

    def to_np(outs):
        full = np.empty((N_TOTAL, 4), np.float32)
        arr = np.asarray(outs[out_names.index("out")]).reshape(NCORES, 4, BCORE)
        for c in range(NCORES):
            full[c * BCORE:(c + 1) * BCORE] = arr[c].T
        return full

    return run, fresh_zeros, to_np

